# revision 16
# baseline (speedup 1.0000x reference)
"""MoE clustered attention kernel for Trainium2 (8 NeuronCores), v2.

Problem: B=2, LQ=LK=2048, D=1024, H=16 heads (DH=64), M=8 clusters.
Each query/key token is routed (argmax of X @ Wr) to one of 8 clusters;
attention is only computed within a cluster (block-sparse attention).

Host: fp32 router argmax (matches the jax reference; top-2 logit gaps
for these inputs are far above rounding noise), gather tokens by
cluster, zero-pad each cluster to a common cross-batch geometry
(queries to multiples of 4, keys to multiples of 128), pre-transpose
X to [D, L] bf16.  Un-permute + sum the 4 head-group partials per
batch afterwards.

Device (per core = batch * 4 + head_group, 4 heads each), designed
around a gapless TensorE stream (TRN2 PE p-state ramping: any stall
drops the PE from 2.4 GHz to 1.2 GHz for the next 3 us):
  * head-pair packed projections: qT2/kT2 [128, L] tiles hold two
    heads' [64, L] blocks; n=512 matmuls, one [128, 512] PSUM
    evacuation per pair-tile alternating Vector/Scalar engines,
  * no mask rows anywhere: padded key columns of XKT are zero, so
    their scores are 0 and exp = 1; vA's mask block (below) zeroes
    their contribution to both the context and the denominator;
    padded q columns produce garbage that the host discards,
  * vA [128 keys, chunk, head, 128]: cols 0:64 a per-key 0/1 mask
    replicated 64x (memset 1.0 then one per-chunk tensor_scalar
    multiply by MSK), cols 64:128 projected values.  The ctx matmul
    (m=128) then yields rows 0:63 = 64 broadcast copies of the
    softmax denominator (at partition base 0, which the custom-DVE
    reciprocal requires) and rows 64:127 = unnormalized context, so
    normalization is reciprocal_approx_fast([64, nq]) + one
    tensor_tensor multiply straight out of PSUM -- no DMA staging,
    no gpsimd broadcast, no ACT table switches,
  * V-projection and attention interleaved in emission so ScalarE's
    exp stream hides under the TensorE window; per-block normalize
    emitted deferred-by-one so it never heads the DVE queue early,
  * exp supertiles: scores land in 2-bank [128, 1024] PSUM tiles,
    one strided ACTIVATE per pair of 128-key chunks (exp table
    preloaded by a dummy exp during Q-proj),
  * output projection n=512, bf16 output (host upcasts and sums).
PSUM: 2 proj + 4 scores + 2 ctx banks = 8.
"""

import numpy as np
import ml_dtypes

import concourse.bacc as bacc
import concourse.tile as tile
import concourse.mybir as mybir
from concourse.bass_utils import run_bass_kernel_spmd

F32 = mybir.dt.float32
BF16 = mybir.dt.bfloat16
EXP = mybir.ActivationFunctionType.Exp
MULT = mybir.AluOpType.mult

H = 16            # total heads
HPC = 4           # heads per core
N_CORES = 8
D = 1024
ND = D // 128


def _ceil_to(x, m):
    return (x + m - 1) // m * m


def _plan(aq, ak, M):
    """Common (cross-batch) padded cluster geometry."""
    B = aq.shape[0]
    nq = np.array([[int((aq[b] == c).sum()) for c in range(M)] for b in range(B)])
    nk = np.array([[int((ak[b] == c).sum()) for c in range(M)] for b in range(B)])
    NQP = [_ceil_to(int(nq[:, c].max()), 4) for c in range(M)]
    NKP = [_ceil_to(max(1, int(nk[:, c].max())), 128) for c in range(M)]
    qoff = np.concatenate([[0], np.cumsum(NQP)])
    koff = np.concatenate([[0], np.cumsum(NKP)])
    return NQP, NKP, qoff[:-1].tolist(), koff[:-1].tolist(), int(qoff[-1]), int(koff[-1])


def _build_program(NQP, NKP, qoffs, koffs, ckoffs, nkmax, LQG, NKG, KL):
    nc = bacc.Bacc("TRN2", target_bir_lowering=False, debug=False)
    TQ = (LQG + 511) // 512
    TK = (KL + 511) // 512
    TV = (NKG + 511) // 512
    # X stays [D, L] (1KB-run descriptors spread partitions across SBUF
    # ports; an 8KB-per-partition layout measurably stalls PE SBUF reads);
    # weights are swizzled to per-partition-contiguous (cheap one-shot loads)
    XQT = nc.dram_tensor("XQT", [D, TQ * 512], BF16, kind="ExternalInput").ap()
    XKT = nc.dram_tensor("XKT", [D, TK * 512], BF16, kind="ExternalInput").ap()
    XVT = nc.dram_tensor("XVT", [D, TV * 512], BF16, kind="ExternalInput").ap()
    WQ = nc.dram_tensor("WQ", [128, ND, 256], BF16, kind="ExternalInput").ap()
    WK = nc.dram_tensor("WK", [128, ND, 256], BF16, kind="ExternalInput").ap()
    WV = nc.dram_tensor("WV", [128, ND, 256], BF16, kind="ExternalInput").ap()
    WO = nc.dram_tensor("WO", [128, 2, 1024], BF16, kind="ExternalInput").ap()
    MSK = nc.dram_tensor("MSK", [128, NKG // 128], F32, kind="ExternalInput").ap()
    OUT = nc.dram_tensor("OUT", [LQG, D], BF16, kind="ExternalOutput").ap()

    NVC = NKG // 128
    M = len(NQP)
    # attention blocks: (cluster, qo, nq) with nq <= 512
    blocks = []
    for c in range(M):
        for qs in range(0, NQP[c], 512):
            blocks.append((c, qoffs[c] + qs, min(512, NQP[c] - qs)))

    with tile.TileContext(nc) as tc:
        with (
            tc.tile_pool(name="weights", bufs=1) as wpool,
            tc.tile_pool(name="big", bufs=1) as bigpool,
            tc.tile_pool(name="xin", bufs=5) as xpool,
            tc.tile_pool(name="es", bufs=4) as espool,
            tc.tile_pool(name="rbt", bufs=3) as rbtpool,
            tc.tile_pool(name="psP", bufs=2, space="PSUM") as psP,
            tc.tile_pool(name="psB", bufs=2, space="PSUM") as psB,
            tc.tile_pool(name="psC", bufs=2, space="PSUM") as psC,
        ):
            wq = wpool.tile([128, ND, 256], BF16, tag="wq")
            wk = wpool.tile([128, ND, 256], BF16, tag="wk")
            wv = wpool.tile([128, ND, 256], BF16, tag="wv")
            wo = wpool.tile([128, 2, 1024], BF16, tag="wo")
            msk = wpool.tile([128, NVC], F32, tag="msk")
            dummy = wpool.tile([1, 8], F32, tag="dummy")

            qT2 = [bigpool.tile([128, LQG], BF16, tag=f"qT{p}", name=f"qT{p}")
                   for p in range(2)]
            kTc = [bigpool.tile([128, KL], BF16, tag=f"kc{p}", name=f"kc{p}")
                   for p in range(2)]
            kT2 = [bigpool.tile([128, NKG], BF16, tag=f"kT{p}", name=f"kT{p}")
                   for p in range(2)]
            vA = bigpool.tile([128, NVC, HPC, 128], BF16, tag="vA")
            ctxT2 = [bigpool.tile([128, LQG], BF16, tag=f"cT{p}", name=f"cT{p}")
                     for p in range(2)]

            nc.sync.dma_start(wq[:], WQ)
            xt0 = xpool.tile([128, ND, 512], BF16, tag="xt", name="xt0")
            XQR = XQT.rearrange("(n p) m -> p n m", p=128)
            nc.sync.dma_start(xt0[:, 0:4, :], XQR[:, 0:4, 0:512])
            nc.scalar.dma_start(xt0[:, 4:8, :], XQR[:, 4:8, 0:512])
            def xt_dma(xt, xdram, t):
                nc.sync.dma_start(
                    xt[:],
                    xdram.rearrange("(n p) m -> p n m", p=128)[:, :, t * 512:(t + 1) * 512])

            # preload the exp ACT table off the critical path
            nc.vector.memset(dummy[:], 0.0)
            nc.scalar.activation(dummy[:], dummy[:], EXP)

            # vA mask block: memset 1.0; padded keys zeroed per chunk in the
            # V loop.  Big memsets run on the idle GpSimd engine so they never
            # block the DVE queue during early projection evacuations.
            nc.gpsimd.memset(vA[:, :, :, 0:64], 1.0)

            # ---- Q/K projections (transposed, head-pair packed) ----
            def proj_T(xdram, L, wtile, dest, evac_flip, first_tile=None,
                       hooks=None):
                for off in range(0, L, 512):
                    if hooks and off // 512 in hooks:
                        hooks[off // 512]()
                    w = min(512, L - off)
                    if off == 0 and first_tile is not None:
                        xt = first_tile
                    else:
                        xt = xpool.tile([128, ND, 512], BF16, tag="xt")
                        xt_dma(xt, xdram, off // 512)
                    for pair in range(2):
                        ps = psP.tile([128, 512], F32, tag="psp")
                        for d in range(ND):
                            nc.tensor.matmul(
                                ps[:, :w],
                                wtile[:, d, pair * 128:(pair + 1) * 128],
                                xt[:, d, :w],
                                start=(d == 0), stop=(d == ND - 1))
                        if evac_flip[0]:
                            nc.vector.tensor_copy(dest[pair][:, off:off + w], ps[:, :w])
                        else:
                            nc.scalar.copy(dest[pair][:, off:off + w], ps[:, :w])
                        evac_flip[0] = not evac_flip[0]

            flip = [True]
            # weight loads deferred into the Q-proj stream (via the ACT HWDGE)
            # so they don't contend with the first xt tiles for DMA bandwidth
            whooks = {
                1: lambda: nc.scalar.dma_start(wk[:], WK),
                2: lambda: nc.scalar.dma_start(wv[:], WV),
                3: lambda: (nc.scalar.dma_start(wo[:], WO),
                            nc.scalar.dma_start(msk[:], MSK)),
            }
            proj_T(XQT, LQG, wq, qT2, flip, first_tile=xt0, hooks=whooks)
            # zero kT2 early (padded key cols must read 0 -> exp = 1, masked)
            nc.gpsimd.memset(kT2[0][:], 0.0)
            nc.gpsimd.memset(kT2[1][:], 0.0)
            proj_T(XKT, KL, wk, kTc, flip)
            M_ = len(NQP)
            for c in range(M_):
                for pair in range(2):
                    w = nkmax[c]
                    if flip[0]:
                        nc.vector.tensor_copy(kT2[pair][:, koffs[c]:koffs[c] + w],
                                              kTc[pair][:, ckoffs[c]:ckoffs[c] + w])
                    else:
                        nc.scalar.copy(kT2[pair][:, koffs[c]:koffs[c] + w],
                                       kTc[pair][:, ckoffs[c]:ckoffs[c] + w])
                    flip[0] = not flip[0]

            # ---- V projection + attention, interleaved emission ----
            def emit_vtile(t):
                lo = t * 512
                w = min(512, NKG - lo)
                xt = xpool.tile([128, ND, 512], BF16, tag="xt")
                xt_dma(xt, XVT, t)
                for sub in range(w // 128):
                    kc = t * 4 + sub
                    ps = psP.tile([128, 256], F32, tag="psp")
                    for d in range(ND):
                        nc.tensor.matmul(ps[:],
                                         xt[:, d, sub * 128:(sub + 1) * 128],
                                         wv[:, d, :],
                                         start=(d == 0), stop=(d == ND - 1))
                    nc.vector.tensor_copy(
                        vA[:, kc, :, 64:128],
                        ps[:].rearrange("p (h e) -> p h e", h=HPC))
                    nc.vector.tensor_scalar_mul(
                        vA[:, kc, :, 0:64], vA[:, kc, :, 0:64], msk[:, kc:kc + 1])

            # software-pipelined emission: iteration i emits scores+exp of
            # unit i, ctx of unit i-1, normalize of unit i-2 -- so the Tensor
            # stream never waits on the exp latency or the normalize chain.
            def emit_scores(u):
                c, qo, nq, h = u["c"], u["qo"], u["nq"], u["h"]
                pair, rb = h // 2, (h % 2) * 64
                es_list = []
                for ki in range(0, u["nkc"], 2):
                    nk2 = min(2, u["nkc"] - ki)
                    ps_s = psB.tile([128, 1024], F32, tag="ps_s")
                    e = espool.tile([128, 1024], BF16, tag="e")
                    for kj in range(nk2):
                        ko = koffs[c] + (ki + kj) * 128
                        nc.tensor.matmul(
                            ps_s[:, kj * 512: kj * 512 + nq],
                            kT2[pair][rb:rb + 64, ko:ko + 128],
                            qT2[pair][rb:rb + 64, qo:qo + nq],
                            start=True, stop=True)
                        es_list.append((e, kj * 512))
                    pv = ps_s[:].rearrange("p (b n) -> p b n", b=2)[:, 0:nk2, 0:nq]
                    ev = e[:].rearrange("p (b n) -> p b n", b=2)[:, 0:nk2, 0:nq]
                    nc.scalar.activation(ev, pv, EXP)
                u["es"] = es_list

            def emit_ctx(u):
                nq, h = u["nq"], u["h"]
                ps_c = psC.tile([128, 512], F32, tag="ps_c")
                for ki in range(u["nkc"]):
                    e, ecol = u["es"][ki]
                    nc.tensor.matmul(ps_c[:, :nq],
                                     vA[:, u["kc0"] + ki, h, :],
                                     e[:, ecol:ecol + nq],
                                     start=(ki == 0), stop=(ki == u["nkc"] - 1))
                u["ps_c"] = ps_c

            def emit_norm(u):
                pair, rb = u["h"] // 2, (u["h"] % 2) * 64
                qo, nq, ps_c = u["qo"], u["nq"], u["ps_c"]
                rbt = rbtpool.tile([64, 512], F32, tag="rbt")
                nc.vector.reciprocal_approx_fast(rbt[:, :nq], ps_c[0:64, :nq])
                nc.vector.tensor_tensor(ctxT2[pair][rb:rb + 64, qo:qo + nq],
                                        ps_c[64:128, :nq], rbt[:, :nq], MULT)

            units = []
            for c, qo, nq in blocks:
                for h in range(HPC):
                    units.append({"c": c, "qo": qo, "nq": nq, "h": h,
                                  "kc0": koffs[c] // 128, "nkc": NKP[c] // 128})

            nvt = (NKG + 511) // 512
            vt_emitted = 0
            for i, u in enumerate(units):
                need = u["kc0"] + u["nkc"]
                while vt_emitted * 4 < need and vt_emitted < nvt:
                    emit_vtile(vt_emitted)
                    vt_emitted += 1
                emit_scores(u)
                if i >= 1:
                    emit_ctx(units[i - 1])
                if i >= 2:
                    emit_norm(units[i - 2])
            emit_ctx(units[-1])
            emit_norm(units[-2])
            emit_norm(units[-1])

        # ---- output projection ----
        NMI = (LQG + 127) // 128
        OUTR = OUT.rearrange("(n p) m -> p n m", p=128) if LQG % 128 == 0 else None
        with tc.tile_pool(name="ob", bufs=3) as obpool, \
             tc.tile_pool(name="psO", bufs=4, space="PSUM") as psO:
            oflip = True
            ob = None
            for mi in range(NMI):
                mw = min(128, LQG - mi * 128)
                if mi % 2 == 0:
                    ob = obpool.tile([128, 2, 1024], BF16, tag="ob")
                half = mi % 2
                for n2 in range(2):
                    ps_o = psO.tile([128, 512], F32, tag="ps_o")
                    for pair in range(2):
                        nc.tensor.matmul(
                            ps_o[:mw, :],
                            ctxT2[pair][:, mi * 128: mi * 128 + mw],
                            wo[:, pair, n2 * 512:(n2 + 1) * 512],
                            start=(pair == 0), stop=(pair == 1))
                    if oflip:
                        nc.vector.tensor_copy(ob[:mw, half, n2 * 512:(n2 + 1) * 512],
                                              ps_o[:mw, :])
                    else:
                        nc.scalar.copy(ob[:mw, half, n2 * 512:(n2 + 1) * 512],
                                       ps_o[:mw, :])
                    oflip = not oflip
                last = (mi == NMI - 1)
                if half == 1 or last:
                    m0 = (mi - half) * 128
                    rows = min(256 if half else 128, LQG - m0)
                    nh = half + 1
                    if OUTR is not None and rows == nh * 128:
                        nc.gpsimd.dma_start(OUTR[:, mi - half: mi + 1, :],
                                          ob[:, 0:nh, :])
                    else:
                        for j in range(nh):
                            r = min(128, LQG - (mi - half + j) * 128)
                            nc.gpsimd.dma_start(
                                OUT[(mi - half + j) * 128:(mi - half + j) * 128 + r, :],
                                ob[:r, j, :])

    nc.compile()
    return nc


_CACHE = {}


def run(inputs, trace=False):
    queries = np.asarray(inputs["queries"], np.float32)
    keys = np.asarray(inputs["keys"], np.float32)
    values = np.asarray(inputs["values"], np.float32)
    Wq = np.asarray(inputs["Wq"], np.float32)
    Wk = np.asarray(inputs["Wk"], np.float32)
    Wv = np.asarray(inputs["Wv"], np.float32)
    Wo = np.asarray(inputs["Wo"], np.float32)
    Wr = np.asarray(inputs["Wr"], np.float32)

    B, LQ, D_ = queries.shape
    M = Wr.shape[1]
    DH = D_ // H
    scale = np.float32(1.0 / np.sqrt(DH))
    npdt = ml_dtypes.bfloat16

    aq = np.argmax(queries @ Wr, axis=-1)   # [B, LQ]
    ak = np.argmax(keys @ Wr, axis=-1)      # [B, LK]

    NQP, NKP, qoffs, koffs, LQG, NKG = _plan(aq, ak, M)
    NVC = NKG // 128
    nk = np.array([[int((ak[b] == c).sum()) for c in range(M)] for b in range(B)])
    nkmax = [int(nk[:, c].max()) for c in range(M)]
    ckoff = np.concatenate([[0], np.cumsum(nkmax)])
    ckoffs = ckoff[:-1].tolist()
    KL = _ceil_to(int(ckoff[-1]), 4)

    key = (tuple(NQP), tuple(NKP), LQG, NKG, KL, tuple(nkmax))
    if key not in _CACHE:
        _CACHE[key] = _build_program(NQP, NKP, qoffs, koffs, ckoffs, nkmax,
                                     LQG, NKG, KL)
    nc = _CACHE[key]

    # ---- gather + zero-pad, build per-batch inputs ----
    perm_q, slot_q = [], []
    XQTs, XKTs, XVTs, MSKs = [], [], [], []
    for b in range(B):
        xq = np.zeros((LQG, D_), np.float32)
        xk = np.zeros((KL, D_), np.float32)
        xv = np.zeros((NKG, D_), np.float32)
        mska = np.zeros((NVC * 128,), np.float32)
        pq, sq = [], []
        for c in range(M):
            tq = np.nonzero(aq[b] == c)[0]
            tk = np.nonzero(ak[b] == c)[0]
            xq[qoffs[c]:qoffs[c] + len(tq)] = queries[b, tq]
            xk[ckoffs[c]:ckoffs[c] + len(tk)] = keys[b, tk]
            xv[koffs[c]:koffs[c] + len(tk)] = values[b, tk]
            mska[koffs[c]:koffs[c] + len(tk)] = 1.0
            pq.append(tq)
            sq.append(np.arange(qoffs[c], qoffs[c] + len(tq)))
        perm_q.append(np.concatenate(pq))
        slot_q.append(np.concatenate(sq))

        def swz(x):
            L = x.shape[0]
            T = (L + 511) // 512
            xp = np.zeros((T * 512, D_), np.float32)
            xp[:L] = x
            return np.ascontiguousarray(xp.T).astype(npdt)

        XQTs.append(swz(xq))
        XKTs.append(swz(xk))
        XVTs.append(swz(xv))
        # msk[p, c] = real(key at chunk c, partition p)
        MSKs.append(np.ascontiguousarray(
            mska.reshape(NVC, 128).T))

    in_maps = []
    for core in range(N_CORES):
        b, hg = core // HPC, core % HPC
        cols = slice(hg * HPC * DH, (hg + 1) * HPC * DH)
        def wswz(w):  # [1024, 256] -> [128, 8, 256]
            return np.ascontiguousarray(
                w.reshape(ND, 128, 256).transpose(1, 0, 2)).astype(npdt)

        in_maps.append({
            "XQT": XQTs[b], "XKT": XKTs[b], "XVT": XVTs[b],
            "WQ": wswz(Wq[:, cols] * scale),
            "WK": wswz(Wk[:, cols]),
            "WV": wswz(Wv[:, cols]),
            "WO": np.ascontiguousarray(
                Wo[cols, :].reshape(2, 128, 1024).transpose(1, 0, 2)).astype(npdt),
            "MSK": MSKs[b],
        })

    res = run_bass_kernel_spmd(nc, in_maps, list(range(N_CORES)), trace=trace)

    out = np.zeros((B, LQ, D_), np.float32)
    for b in range(B):
        acc = res.results[b * HPC]["OUT"].astype(np.float32)
        for hg in range(1, HPC):
            acc += res.results[b * HPC + hg]["OUT"].astype(np.float32)
        out[b, perm_q[b]] = acc[slot_q[b]]
    return out, res


def kernel(**inputs):
    out, _ = run(inputs)
    return out


# revision 17
# speedup vs baseline: 1.0000x; 1.0000x over previous
"""MoE clustered attention kernel for Trainium2 (8 NeuronCores), v2.

Problem: B=2, LQ=LK=2048, D=1024, H=16 heads (DH=64), M=8 clusters.
Each query/key token is routed (argmax of X @ Wr) to one of 8 clusters;
attention is only computed within a cluster (block-sparse attention).

Host: fp32 router argmax (matches the jax reference; top-2 logit gaps
for these inputs are far above rounding noise), gather tokens by
cluster, zero-pad each cluster to a common cross-batch geometry
(queries to multiples of 4, keys to multiples of 128), pre-transpose
X to [D, L] bf16.  Un-permute + sum the 4 head-group partials per
batch afterwards.

Device (per core = batch * 4 + head_group, 4 heads each), designed
around a gapless TensorE stream (TRN2 PE p-state ramping: any stall
drops the PE from 2.4 GHz to 1.2 GHz for the next 3 us):
  * head-pair packed projections: qT2/kT2 [128, L] tiles hold two
    heads' [64, L] blocks; n=512 matmuls, one [128, 512] PSUM
    evacuation per pair-tile alternating Vector/Scalar engines,
  * no mask rows anywhere: padded key columns of XKT are zero, so
    their scores are 0 and exp = 1; vA's mask block (below) zeroes
    their contribution to both the context and the denominator;
    padded q columns produce garbage that the host discards,
  * vA [128 keys, chunk, head, 128]: cols 0:64 a per-key 0/1 mask
    replicated 64x (memset 1.0 then one per-chunk tensor_scalar
    multiply by MSK), cols 64:128 projected values.  The ctx matmul
    (m=128) then yields rows 0:63 = 64 broadcast copies of the
    softmax denominator (at partition base 0, which the custom-DVE
    reciprocal requires) and rows 64:127 = unnormalized context, so
    normalization is reciprocal_approx_fast([64, nq]) + one
    tensor_tensor multiply straight out of PSUM -- no DMA staging,
    no gpsimd broadcast, no ACT table switches,
  * V-projection and attention interleaved in emission so ScalarE's
    exp stream hides under the TensorE window; per-block normalize
    emitted deferred-by-one so it never heads the DVE queue early,
  * exp supertiles: scores land in 2-bank [128, 1024] PSUM tiles,
    one strided ACTIVATE per pair of 128-key chunks (exp table
    preloaded by a dummy exp during Q-proj),
  * output projection n=512, bf16 output (host upcasts and sums).
PSUM: 2 proj + 4 scores + 2 ctx banks = 8.
"""

import numpy as np
import ml_dtypes

import concourse.bacc as bacc
import concourse.tile as tile
import concourse.mybir as mybir
from concourse.bass_utils import run_bass_kernel_spmd

F32 = mybir.dt.float32
BF16 = mybir.dt.bfloat16
EXP = mybir.ActivationFunctionType.Exp
MULT = mybir.AluOpType.mult

H = 16            # total heads
HPC = 4           # heads per core
N_CORES = 8
D = 1024
ND = D // 128


def _ceil_to(x, m):
    return (x + m - 1) // m * m


def _plan(aq, ak, M):
    """Common (cross-batch) padded cluster geometry."""
    B = aq.shape[0]
    nq = np.array([[int((aq[b] == c).sum()) for c in range(M)] for b in range(B)])
    nk = np.array([[int((ak[b] == c).sum()) for c in range(M)] for b in range(B)])
    NQP = [_ceil_to(int(nq[:, c].max()), 4) for c in range(M)]
    NKP = [_ceil_to(max(1, int(nk[:, c].max())), 128) for c in range(M)]
    qoff = np.concatenate([[0], np.cumsum(NQP)])
    koff = np.concatenate([[0], np.cumsum(NKP)])
    return NQP, NKP, qoff[:-1].tolist(), koff[:-1].tolist(), int(qoff[-1]), int(koff[-1])


def _build_program(NQP, NKP, qoffs, koffs, ckoffs, nkmax, LQG, NKG, KL):
    nc = bacc.Bacc("TRN2", target_bir_lowering=False, debug=False)
    TQ = (LQG + 511) // 512
    TK = (KL + 511) // 512
    TV = (NKG + 511) // 512
    # X stays [D, L] (1KB-run descriptors spread partitions across SBUF
    # ports; an 8KB-per-partition layout measurably stalls PE SBUF reads);
    # weights are swizzled to per-partition-contiguous (cheap one-shot loads)
    XQT = nc.dram_tensor("XQT", [D, TQ * 512], BF16, kind="ExternalInput").ap()
    XQ0 = nc.dram_tensor("XQ0", [128, 2, ND, 512], BF16, kind="ExternalInput").ap()
    XKT = nc.dram_tensor("XKT", [D, TK * 512], BF16, kind="ExternalInput").ap()
    XVT = nc.dram_tensor("XVT", [D, TV * 512], BF16, kind="ExternalInput").ap()
    WQ = nc.dram_tensor("WQ", [128, ND, 256], BF16, kind="ExternalInput").ap()
    WK = nc.dram_tensor("WK", [128, ND, 256], BF16, kind="ExternalInput").ap()
    WV = nc.dram_tensor("WV", [128, ND, 256], BF16, kind="ExternalInput").ap()
    WO = nc.dram_tensor("WO", [128, 2, 1024], BF16, kind="ExternalInput").ap()
    MSK = nc.dram_tensor("MSK", [128, NKG // 128], F32, kind="ExternalInput").ap()
    OUT = nc.dram_tensor("OUT", [LQG, D], BF16, kind="ExternalOutput").ap()

    NVC = NKG // 128
    M = len(NQP)
    # attention blocks: (cluster, qo, nq) with nq <= 512
    blocks = []
    for c in range(M):
        for qs in range(0, NQP[c], 512):
            blocks.append((c, qoffs[c] + qs, min(512, NQP[c] - qs)))

    with tile.TileContext(nc) as tc:
        with (
            tc.tile_pool(name="weights", bufs=1) as wpool,
            tc.tile_pool(name="big", bufs=1) as bigpool,
            tc.tile_pool(name="xin", bufs=5) as xpool,
            tc.tile_pool(name="es", bufs=4) as espool,
            tc.tile_pool(name="rbt", bufs=3) as rbtpool,
            tc.tile_pool(name="psP", bufs=2, space="PSUM") as psP,
            tc.tile_pool(name="psB", bufs=2, space="PSUM") as psB,
            tc.tile_pool(name="psC", bufs=2, space="PSUM") as psC,
        ):
            wq = wpool.tile([128, ND, 256], BF16, tag="wq")
            wk = wpool.tile([128, ND, 256], BF16, tag="wk")
            wv = wpool.tile([128, ND, 256], BF16, tag="wv")
            wo = wpool.tile([128, 2, 1024], BF16, tag="wo")
            msk = wpool.tile([128, NVC], F32, tag="msk")
            dummy = wpool.tile([1, 8], F32, tag="dummy")

            qT2 = [bigpool.tile([128, LQG], BF16, tag=f"qT{p}", name=f"qT{p}")
                   for p in range(2)]
            kTc = [bigpool.tile([128, KL], BF16, tag=f"kc{p}", name=f"kc{p}")
                   for p in range(2)]
            kT2 = [bigpool.tile([128, NKG], BF16, tag=f"kT{p}", name=f"kT{p}")
                   for p in range(2)]
            vA = bigpool.tile([128, NVC, HPC, 128], BF16, tag="vA")
            ctxT2 = [bigpool.tile([128, LQG], BF16, tag=f"cT{p}", name=f"cT{p}")
                     for p in range(2)]

            nc.sync.dma_start(wq[:], WQ)
            # first two Q tiles come pre-swizzled (128 descriptors each, ~8x
            # faster to generate); their transfers land before compute starts
            # so the per-partition burst pattern can't stall PE SBUF reads
            xt0 = xpool.tile([128, ND, 512], BF16, tag="xt", name="xt0")
            nc.sync.dma_start(xt0[:], XQ0[:, 0])
            xt1 = xpool.tile([128, ND, 512], BF16, tag="xt", name="xt1")
            nc.scalar.dma_start(xt1[:], XQ0[:, 1])
            def xt_dma(xt, xdram, t):
                nc.sync.dma_start(
                    xt[:],
                    xdram.rearrange("(n p) m -> p n m", p=128)[:, :, t * 512:(t + 1) * 512])

            # preload the exp ACT table off the critical path
            nc.vector.memset(dummy[:], 0.0)
            nc.scalar.activation(dummy[:], dummy[:], EXP)

            # vA mask block: memset 1.0; padded keys zeroed per chunk in the
            # V loop.  Big memsets run on the idle GpSimd engine so they never
            # block the DVE queue during early projection evacuations.
            nc.gpsimd.memset(vA[:, :, :, 0:64], 1.0)

            # ---- Q/K projections (transposed, head-pair packed) ----
            def proj_T(xdram, L, wtile, dest, evac_flip, first_tile=None,
                       hooks=None):
                for off in range(0, L, 512):
                    if hooks and off // 512 in hooks:
                        hooks[off // 512]()
                    w = min(512, L - off)
                    if first_tile is not None and off // 512 < len(first_tile):
                        xt = first_tile[off // 512]
                    else:
                        xt = xpool.tile([128, ND, 512], BF16, tag="xt")
                        xt_dma(xt, xdram, off // 512)
                    for pair in range(2):
                        ps = psP.tile([128, 512], F32, tag="psp")
                        for d in range(ND):
                            nc.tensor.matmul(
                                ps[:, :w],
                                wtile[:, d, pair * 128:(pair + 1) * 128],
                                xt[:, d, :w],
                                start=(d == 0), stop=(d == ND - 1))
                        if evac_flip[0]:
                            nc.vector.tensor_copy(dest[pair][:, off:off + w], ps[:, :w])
                        else:
                            nc.scalar.copy(dest[pair][:, off:off + w], ps[:, :w])
                        evac_flip[0] = not evac_flip[0]

            flip = [True]
            # weight loads deferred into the Q-proj stream (via the ACT HWDGE)
            # so they don't contend with the first xt tiles for DMA bandwidth
            whooks = {
                1: lambda: nc.scalar.dma_start(wk[:], WK),
                2: lambda: nc.scalar.dma_start(wv[:], WV),
                3: lambda: (nc.scalar.dma_start(wo[:], WO),
                            nc.scalar.dma_start(msk[:], MSK)),
            }
            proj_T(XQT, LQG, wq, qT2, flip, first_tile=[xt0, xt1], hooks=whooks)
            # zero kT2 early (padded key cols must read 0 -> exp = 1, masked)
            nc.gpsimd.memset(kT2[0][:], 0.0)
            nc.gpsimd.memset(kT2[1][:], 0.0)
            proj_T(XKT, KL, wk, kTc, flip)
            M_ = len(NQP)
            for c in range(M_):
                for pair in range(2):
                    w = nkmax[c]
                    if flip[0]:
                        nc.vector.tensor_copy(kT2[pair][:, koffs[c]:koffs[c] + w],
                                              kTc[pair][:, ckoffs[c]:ckoffs[c] + w])
                    else:
                        nc.scalar.copy(kT2[pair][:, koffs[c]:koffs[c] + w],
                                       kTc[pair][:, ckoffs[c]:ckoffs[c] + w])
                    flip[0] = not flip[0]

            # ---- V projection + attention, interleaved emission ----
            def emit_vtile(t):
                lo = t * 512
                w = min(512, NKG - lo)
                xt = xpool.tile([128, ND, 512], BF16, tag="xt")
                xt_dma(xt, XVT, t)
                for sub in range(w // 128):
                    kc = t * 4 + sub
                    ps = psP.tile([128, 256], F32, tag="psp")
                    for d in range(ND):
                        nc.tensor.matmul(ps[:],
                                         xt[:, d, sub * 128:(sub + 1) * 128],
                                         wv[:, d, :],
                                         start=(d == 0), stop=(d == ND - 1))
                    nc.vector.tensor_copy(
                        vA[:, kc, :, 64:128],
                        ps[:].rearrange("p (h e) -> p h e", h=HPC))
                    nc.vector.tensor_scalar_mul(
                        vA[:, kc, :, 0:64], vA[:, kc, :, 0:64], msk[:, kc:kc + 1])

            # software-pipelined emission: iteration i emits scores+exp of
            # unit i, ctx of unit i-1, normalize of unit i-2 -- so the Tensor
            # stream never waits on the exp latency or the normalize chain.
            def emit_scores(u):
                c, qo, nq, h = u["c"], u["qo"], u["nq"], u["h"]
                pair, rb = h // 2, (h % 2) * 64
                es_list = []
                for ki in range(0, u["nkc"], 2):
                    nk2 = min(2, u["nkc"] - ki)
                    ps_s = psB.tile([128, 1024], F32, tag="ps_s")
                    e = espool.tile([128, 1024], BF16, tag="e")
                    for kj in range(nk2):
                        ko = koffs[c] + (ki + kj) * 128
                        nc.tensor.matmul(
                            ps_s[:, kj * 512: kj * 512 + nq],
                            kT2[pair][rb:rb + 64, ko:ko + 128],
                            qT2[pair][rb:rb + 64, qo:qo + nq],
                            start=True, stop=True)
                        es_list.append((e, kj * 512))
                    pv = ps_s[:].rearrange("p (b n) -> p b n", b=2)[:, 0:nk2, 0:nq]
                    ev = e[:].rearrange("p (b n) -> p b n", b=2)[:, 0:nk2, 0:nq]
                    nc.scalar.activation(ev, pv, EXP)
                u["es"] = es_list

            def emit_ctx(u):
                nq, h = u["nq"], u["h"]
                ps_c = psC.tile([128, 512], F32, tag="ps_c")
                for ki in range(u["nkc"]):
                    e, ecol = u["es"][ki]
                    nc.tensor.matmul(ps_c[:, :nq],
                                     vA[:, u["kc0"] + ki, h, :],
                                     e[:, ecol:ecol + nq],
                                     start=(ki == 0), stop=(ki == u["nkc"] - 1))
                u["ps_c"] = ps_c

            def emit_norm(u):
                pair, rb = u["h"] // 2, (u["h"] % 2) * 64
                qo, nq, ps_c = u["qo"], u["nq"], u["ps_c"]
                rbt = rbtpool.tile([64, 512], F32, tag="rbt")
                nc.vector.reciprocal_approx_fast(rbt[:, :nq], ps_c[0:64, :nq])
                nc.vector.tensor_tensor(ctxT2[pair][rb:rb + 64, qo:qo + nq],
                                        ps_c[64:128, :nq], rbt[:, :nq], MULT)

            units = []
            for c, qo, nq in blocks:
                for h in range(HPC):
                    units.append({"c": c, "qo": qo, "nq": nq, "h": h,
                                  "kc0": koffs[c] // 128, "nkc": NKP[c] // 128})

            nvt = (NKG + 511) // 512
            vt_emitted = 0
            for i, u in enumerate(units):
                need = u["kc0"] + u["nkc"]
                while vt_emitted * 4 < need and vt_emitted < nvt:
                    emit_vtile(vt_emitted)
                    vt_emitted += 1
                emit_scores(u)
                if i >= 1:
                    emit_ctx(units[i - 1])
                if i >= 2:
                    emit_norm(units[i - 2])
            emit_ctx(units[-1])
            emit_norm(units[-2])
            emit_norm(units[-1])

        # ---- output projection ----
        NMI = (LQG + 127) // 128
        OUTR = OUT.rearrange("(n p) m -> p n m", p=128) if LQG % 128 == 0 else None
        with tc.tile_pool(name="ob", bufs=3) as obpool, \
             tc.tile_pool(name="psO", bufs=4, space="PSUM") as psO:
            oflip = True
            ob = None
            for mi in range(NMI):
                mw = min(128, LQG - mi * 128)
                if mi % 2 == 0:
                    ob = obpool.tile([128, 2, 1024], BF16, tag="ob")
                half = mi % 2
                for n2 in range(2):
                    ps_o = psO.tile([128, 512], F32, tag="ps_o")
                    for pair in range(2):
                        nc.tensor.matmul(
                            ps_o[:mw, :],
                            ctxT2[pair][:, mi * 128: mi * 128 + mw],
                            wo[:, pair, n2 * 512:(n2 + 1) * 512],
                            start=(pair == 0), stop=(pair == 1))
                    if oflip:
                        nc.vector.tensor_copy(ob[:mw, half, n2 * 512:(n2 + 1) * 512],
                                              ps_o[:mw, :])
                    else:
                        nc.scalar.copy(ob[:mw, half, n2 * 512:(n2 + 1) * 512],
                                       ps_o[:mw, :])
                    oflip = not oflip
                last = (mi == NMI - 1)
                if half == 1 or last:
                    m0 = (mi - half) * 128
                    rows = min(256 if half else 128, LQG - m0)
                    nh = half + 1
                    if OUTR is not None and rows == nh * 128:
                        nc.gpsimd.dma_start(OUTR[:, mi - half: mi + 1, :],
                                          ob[:, 0:nh, :])
                    else:
                        for j in range(nh):
                            r = min(128, LQG - (mi - half + j) * 128)
                            nc.gpsimd.dma_start(
                                OUT[(mi - half + j) * 128:(mi - half + j) * 128 + r, :],
                                ob[:r, j, :])

    nc.compile()
    return nc


_CACHE = {}


def run(inputs, trace=False):
    queries = np.asarray(inputs["queries"], np.float32)
    keys = np.asarray(inputs["keys"], np.float32)
    values = np.asarray(inputs["values"], np.float32)
    Wq = np.asarray(inputs["Wq"], np.float32)
    Wk = np.asarray(inputs["Wk"], np.float32)
    Wv = np.asarray(inputs["Wv"], np.float32)
    Wo = np.asarray(inputs["Wo"], np.float32)
    Wr = np.asarray(inputs["Wr"], np.float32)

    B, LQ, D_ = queries.shape
    M = Wr.shape[1]
    DH = D_ // H
    scale = np.float32(1.0 / np.sqrt(DH))
    npdt = ml_dtypes.bfloat16

    aq = np.argmax(queries @ Wr, axis=-1)   # [B, LQ]
    ak = np.argmax(keys @ Wr, axis=-1)      # [B, LK]

    NQP, NKP, qoffs, koffs, LQG, NKG = _plan(aq, ak, M)
    NVC = NKG // 128
    nk = np.array([[int((ak[b] == c).sum()) for c in range(M)] for b in range(B)])
    nkmax = [int(nk[:, c].max()) for c in range(M)]
    ckoff = np.concatenate([[0], np.cumsum(nkmax)])
    ckoffs = ckoff[:-1].tolist()
    KL = _ceil_to(int(ckoff[-1]), 4)

    key = (tuple(NQP), tuple(NKP), LQG, NKG, KL, tuple(nkmax))
    if key not in _CACHE:
        _CACHE[key] = _build_program(NQP, NKP, qoffs, koffs, ckoffs, nkmax,
                                     LQG, NKG, KL)
    nc = _CACHE[key]

    # ---- gather + zero-pad, build per-batch inputs ----
    perm_q, slot_q = [], []
    XQTs, XKTs, XVTs, MSKs = [], [], [], []
    for b in range(B):
        xq = np.zeros((LQG, D_), np.float32)
        xk = np.zeros((KL, D_), np.float32)
        xv = np.zeros((NKG, D_), np.float32)
        mska = np.zeros((NVC * 128,), np.float32)
        pq, sq = [], []
        for c in range(M):
            tq = np.nonzero(aq[b] == c)[0]
            tk = np.nonzero(ak[b] == c)[0]
            xq[qoffs[c]:qoffs[c] + len(tq)] = queries[b, tq]
            xk[ckoffs[c]:ckoffs[c] + len(tk)] = keys[b, tk]
            xv[koffs[c]:koffs[c] + len(tk)] = values[b, tk]
            mska[koffs[c]:koffs[c] + len(tk)] = 1.0
            pq.append(tq)
            sq.append(np.arange(qoffs[c], qoffs[c] + len(tq)))
        perm_q.append(np.concatenate(pq))
        slot_q.append(np.concatenate(sq))

        def swz(x):
            L = x.shape[0]
            T = (L + 511) // 512
            xp = np.zeros((T * 512, D_), np.float32)
            xp[:L] = x
            return np.ascontiguousarray(xp.T).astype(npdt)

        XQTs.append(swz(xq))
        XKTs.append(swz(xk))
        XVTs.append(swz(xv))
        # msk[p, c] = real(key at chunk c, partition p)
        MSKs.append(np.ascontiguousarray(
            mska.reshape(NVC, 128).T))

    in_maps = []
    for core in range(N_CORES):
        b, hg = core // HPC, core % HPC
        cols = slice(hg * HPC * DH, (hg + 1) * HPC * DH)
        def wswz(w):  # [1024, 256] -> [128, 8, 256]
            return np.ascontiguousarray(
                w.reshape(ND, 128, 256).transpose(1, 0, 2)).astype(npdt)

        xq0 = np.ascontiguousarray(
            XQTs[b][:, :1024].reshape(ND, 128, 2, 512).transpose(1, 2, 0, 3))
        in_maps.append({
            "XQT": XQTs[b], "XKT": XKTs[b], "XVT": XVTs[b], "XQ0": xq0,
            "WQ": wswz(Wq[:, cols] * scale),
            "WK": wswz(Wk[:, cols]),
            "WV": wswz(Wv[:, cols]),
            "WO": np.ascontiguousarray(
                Wo[cols, :].reshape(2, 128, 1024).transpose(1, 0, 2)).astype(npdt),
            "MSK": MSKs[b],
        })

    res = run_bass_kernel_spmd(nc, in_maps, list(range(N_CORES)), trace=trace)

    out = np.zeros((B, LQ, D_), np.float32)
    for b in range(B):
        acc = res.results[b * HPC]["OUT"].astype(np.float32)
        for hg in range(1, HPC):
            acc += res.results[b * HPC + hg]["OUT"].astype(np.float32)
        out[b, perm_q[b]] = acc[slot_q[b]]
    return out, res


def kernel(**inputs):
    out, _ = run(inputs)
    return out


# revision 19
# speedup vs baseline: 1.0070x; 1.0069x over previous
"""MoE clustered attention kernel for Trainium2 (8 NeuronCores), v2.

Problem: B=2, LQ=LK=2048, D=1024, H=16 heads (DH=64), M=8 clusters.
Each query/key token is routed (argmax of X @ Wr) to one of 8 clusters;
attention is only computed within a cluster (block-sparse attention).

Host: fp32 router argmax (matches the jax reference; top-2 logit gaps
for these inputs are far above rounding noise), gather tokens by
cluster, zero-pad each cluster to a common cross-batch geometry
(queries to multiples of 4, keys to multiples of 128), pre-transpose
X to [D, L] bf16.  Un-permute + sum the 4 head-group partials per
batch afterwards.

Device (per core = batch * 4 + head_group, 4 heads each), designed
around a gapless TensorE stream (TRN2 PE p-state ramping: any stall
drops the PE from 2.4 GHz to 1.2 GHz for the next 3 us):
  * head-pair packed projections: qT2/kT2 [128, L] tiles hold two
    heads' [64, L] blocks; n=512 matmuls, one [128, 512] PSUM
    evacuation per pair-tile alternating Vector/Scalar engines,
  * no mask rows anywhere: padded key columns of XKT are zero, so
    their scores are 0 and exp = 1; vA's mask block (below) zeroes
    their contribution to both the context and the denominator;
    padded q columns produce garbage that the host discards,
  * vA [128 keys, chunk, head, 128]: cols 0:64 a per-key 0/1 mask
    replicated 64x (memset 1.0 then one per-chunk tensor_scalar
    multiply by MSK), cols 64:128 projected values.  The ctx matmul
    (m=128) then yields rows 0:63 = 64 broadcast copies of the
    softmax denominator (at partition base 0, which the custom-DVE
    reciprocal requires) and rows 64:127 = unnormalized context, so
    normalization is reciprocal_approx_fast([64, nq]) + one
    tensor_tensor multiply straight out of PSUM -- no DMA staging,
    no gpsimd broadcast, no ACT table switches,
  * V-projection and attention interleaved in emission so ScalarE's
    exp stream hides under the TensorE window; per-block normalize
    emitted deferred-by-one so it never heads the DVE queue early,
  * exp supertiles: scores land in 2-bank [128, 1024] PSUM tiles,
    one strided ACTIVATE per pair of 128-key chunks (exp table
    preloaded by a dummy exp during Q-proj),
  * output projection n=512, bf16 output (host upcasts and sums).
PSUM: 2 proj + 4 scores + 2 ctx banks = 8.
"""

import numpy as np
import ml_dtypes

import concourse.bacc as bacc
import concourse.tile as tile
import concourse.mybir as mybir
from concourse.bass_utils import run_bass_kernel_spmd

F32 = mybir.dt.float32
BF16 = mybir.dt.bfloat16
EXP = mybir.ActivationFunctionType.Exp
MULT = mybir.AluOpType.mult

H = 16            # total heads
HPC = 4           # heads per core
N_CORES = 8
D = 1024
ND = D // 128


def _ceil_to(x, m):
    return (x + m - 1) // m * m


def _plan(aq, ak, M):
    """Common (cross-batch) padded cluster geometry."""
    B = aq.shape[0]
    nq = np.array([[int((aq[b] == c).sum()) for c in range(M)] for b in range(B)])
    nk = np.array([[int((ak[b] == c).sum()) for c in range(M)] for b in range(B)])
    NQP = [_ceil_to(int(nq[:, c].max()), 4) for c in range(M)]
    NKP = [_ceil_to(max(1, int(nk[:, c].max())), 128) for c in range(M)]
    qoff = np.concatenate([[0], np.cumsum(NQP)])
    koff = np.concatenate([[0], np.cumsum(NKP)])
    return NQP, NKP, qoff[:-1].tolist(), koff[:-1].tolist(), int(qoff[-1]), int(koff[-1])


def _build_program(NQP, NKP, qoffs, koffs, ckoffs, nkmax, LQG, NKG, KL):
    nc = bacc.Bacc("TRN2", target_bir_lowering=False, debug=False)
    TQ = (LQG + 511) // 512
    TK = (KL + 511) // 512
    TV = (NKG + 511) // 512
    # X stays [D, L] (1KB-run descriptors spread partitions across SBUF
    # ports; an 8KB-per-partition layout measurably stalls PE SBUF reads);
    # weights are swizzled to per-partition-contiguous (cheap one-shot loads)
    XQT = nc.dram_tensor("XQT", [D, TQ * 512], BF16, kind="ExternalInput").ap()
    XQ0 = nc.dram_tensor("XQ0", [128, 2, ND, 512], BF16, kind="ExternalInput").ap()
    XKT = nc.dram_tensor("XKT", [D, TK * 512], BF16, kind="ExternalInput").ap()
    XVT = nc.dram_tensor("XVT", [D, TV * 512], BF16, kind="ExternalInput").ap()
    WQ = nc.dram_tensor("WQ", [128, ND, 256], BF16, kind="ExternalInput").ap()
    WK = nc.dram_tensor("WK", [128, ND, 256], BF16, kind="ExternalInput").ap()
    WV = nc.dram_tensor("WV", [128, ND, 256], BF16, kind="ExternalInput").ap()
    WO = nc.dram_tensor("WO", [128, 2, 1024], BF16, kind="ExternalInput").ap()
    MSK = nc.dram_tensor("MSK", [128, NKG // 128], F32, kind="ExternalInput").ap()
    OUT = nc.dram_tensor("OUT", [LQG, D], BF16, kind="ExternalOutput").ap()

    NVC = NKG // 128
    M = len(NQP)
    # attention blocks: (cluster, qo, nq) with nq <= 512
    blocks = []
    for c in range(M):
        for qs in range(0, NQP[c], 512):
            blocks.append((c, qoffs[c] + qs, min(512, NQP[c] - qs)))

    with tile.TileContext(nc) as tc:
        with (
            tc.tile_pool(name="weights", bufs=1) as wpool,
            tc.tile_pool(name="big", bufs=1) as bigpool,
            tc.tile_pool(name="xin", bufs=5) as xpool,
            tc.tile_pool(name="es", bufs=4) as espool,
            tc.tile_pool(name="rbt", bufs=3) as rbtpool,
            tc.tile_pool(name="psP", bufs=2, space="PSUM") as psP,
            tc.tile_pool(name="psB", bufs=2, space="PSUM") as psB,
            tc.tile_pool(name="psC", bufs=2, space="PSUM") as psC,
        ):
            wq = wpool.tile([128, ND, 256], BF16, tag="wq")
            wk = wpool.tile([128, ND, 256], BF16, tag="wk")
            wv = wpool.tile([128, ND, 256], BF16, tag="wv")
            wo = wpool.tile([128, 2, 1024], BF16, tag="wo")
            msk = wpool.tile([128, NVC], F32, tag="msk")
            dummy = wpool.tile([1, 8], F32, tag="dummy")

            qT2 = [bigpool.tile([128, LQG], BF16, tag=f"qT{p}", name=f"qT{p}")
                   for p in range(2)]
            kTc = [bigpool.tile([128, KL], BF16, tag=f"kc{p}", name=f"kc{p}")
                   for p in range(2)]
            kT2 = [bigpool.tile([128, NKG], BF16, tag=f"kT{p}", name=f"kT{p}")
                   for p in range(2)]
            vA = bigpool.tile([128, NVC, HPC, 128], BF16, tag="vA")
            ctxT2 = [bigpool.tile([128, LQG], BF16, tag=f"cT{p}", name=f"cT{p}")
                     for p in range(2)]

            nc.sync.dma_start(wq[:], WQ)
            # first two Q tiles come pre-swizzled (128 descriptors each, ~8x
            # faster to generate); their transfers land before compute starts
            # so the per-partition burst pattern can't stall PE SBUF reads
            xt0 = xpool.tile([128, ND, 512], BF16, tag="xt", name="xt0")
            nc.sync.dma_start(xt0[:], XQ0[:, 0])
            xt1 = xpool.tile([128, ND, 512], BF16, tag="xt", name="xt1")
            nc.scalar.dma_start(xt1[:], XQ0[:, 1])
            def xt_dma(xt, xdram, t):
                nc.sync.dma_start(
                    xt[:],
                    xdram.rearrange("(n p) m -> p n m", p=128)[:, :, t * 512:(t + 1) * 512])

            # preload the exp ACT table off the critical path
            nc.vector.memset(dummy[:], 0.0)
            nc.scalar.activation(dummy[:], dummy[:], EXP)

            # vA mask block: memset 1.0; padded keys zeroed per chunk in the
            # V loop.  Big memsets run on the idle GpSimd engine so they never
            # block the DVE queue during early projection evacuations.
            nc.gpsimd.memset(vA[:, :, :, 0:64], 1.0)

            # ---- Q/K projections (transposed, head-pair packed) ----
            def proj_T(xdram, L, wtile, dest, evac_flip, first_tile=None,
                       hooks=None):
                for off in range(0, L, 512):
                    if hooks and off // 512 in hooks:
                        hooks[off // 512]()
                    w = min(512, L - off)
                    if first_tile is not None and off // 512 < len(first_tile):
                        xt = first_tile[off // 512]
                    else:
                        xt = xpool.tile([128, ND, 512], BF16, tag="xt")
                        xt_dma(xt, xdram, off // 512)
                    for pair in range(2):
                        ps = psP.tile([128, 512], F32, tag="psp")
                        for d in range(ND):
                            nc.tensor.matmul(
                                ps[:, :w],
                                wtile[:, d, pair * 128:(pair + 1) * 128],
                                xt[:, d, :w],
                                start=(d == 0), stop=(d == ND - 1))
                        if evac_flip[0]:
                            nc.vector.tensor_copy(dest[pair][:, off:off + w], ps[:, :w])
                        else:
                            nc.scalar.copy(dest[pair][:, off:off + w], ps[:, :w])
                        evac_flip[0] = not evac_flip[0]

            flip = [True]
            # weight loads deferred into the Q-proj stream (via the ACT HWDGE)
            # so they don't contend with the first xt tiles for DMA bandwidth
            whooks = {
                1: lambda: nc.scalar.dma_start(wk[:], WK),
                2: lambda: nc.scalar.dma_start(wv[:], WV),
                3: lambda: (nc.scalar.dma_start(wo[:], WO),
                            nc.scalar.dma_start(msk[:], MSK)),
            }
            proj_T(XQT, LQG, wq, qT2, flip, first_tile=[xt0, xt1], hooks=whooks)
            # zero kT2 early (padded key cols must read 0 -> exp = 1, masked)
            nc.gpsimd.memset(kT2[0][:], 0.0)
            nc.gpsimd.memset(kT2[1][:], 0.0)
            proj_T(XKT, KL, wk, kTc, flip)
            M_ = len(NQP)
            for c in range(M_):
                for pair in range(2):
                    w = nkmax[c]
                    if flip[0]:
                        nc.vector.tensor_copy(kT2[pair][:, koffs[c]:koffs[c] + w],
                                              kTc[pair][:, ckoffs[c]:ckoffs[c] + w])
                    else:
                        nc.scalar.copy(kT2[pair][:, koffs[c]:koffs[c] + w],
                                       kTc[pair][:, ckoffs[c]:ckoffs[c] + w])
                    flip[0] = not flip[0]

            # ---- V projection + attention, interleaved emission ----
            def emit_vtile(t):
                lo = t * 512
                w = min(512, NKG - lo)
                xt = xpool.tile([128, ND, 512], BF16, tag="xt")
                xt_dma(xt, XVT, t)
                for sub in range(w // 128):
                    kc = t * 4 + sub
                    ps = psP.tile([128, 256], F32, tag="psp")
                    for d in range(ND):
                        nc.tensor.matmul(ps[:],
                                         xt[:, d, sub * 128:(sub + 1) * 128],
                                         wv[:, d, :],
                                         start=(d == 0), stop=(d == ND - 1))
                    nc.vector.tensor_copy(
                        vA[:, kc, :, 64:128],
                        ps[:].rearrange("p (h e) -> p h e", h=HPC))
                    nc.vector.tensor_scalar_mul(
                        vA[:, kc, :, 0:64], vA[:, kc, :, 0:64], msk[:, kc:kc + 1])

            # software-pipelined emission: iteration i emits scores+exp of
            # unit i, ctx of unit i-1, normalize of unit i-2 -- so the Tensor
            # stream never waits on the exp latency or the normalize chain.
            def emit_scores(u):
                c, qo, nq, h = u["c"], u["qo"], u["nq"], u["h"]
                pair, rb = h // 2, (h % 2) * 64
                es_list = []
                for ki in range(0, u["nkc"], 2):
                    nk2 = min(2, u["nkc"] - ki)
                    ps_s = psB.tile([128, 1024], F32, tag="ps_s")
                    e = espool.tile([128, 1024], BF16, tag="e")
                    for kj in range(nk2):
                        ko = koffs[c] + (ki + kj) * 128
                        nc.tensor.matmul(
                            ps_s[:, kj * 512: kj * 512 + nq],
                            kT2[pair][rb:rb + 64, ko:ko + 128],
                            qT2[pair][rb:rb + 64, qo:qo + nq],
                            start=True, stop=True)
                        es_list.append((e, kj * 512))
                    pv = ps_s[:].rearrange("p (b n) -> p b n", b=2)[:, 0:nk2, 0:nq]
                    ev = e[:].rearrange("p (b n) -> p b n", b=2)[:, 0:nk2, 0:nq]
                    nc.scalar.activation(ev, pv, EXP)
                u["es"] = es_list

            def emit_ctx(u):
                nq, h = u["nq"], u["h"]
                ps_c = psC.tile([128, 512], F32, tag="ps_c")
                for ki in range(u["nkc"]):
                    e, ecol = u["es"][ki]
                    nc.tensor.matmul(ps_c[:, :nq],
                                     vA[:, u["kc0"] + ki, h, :],
                                     e[:, ecol:ecol + nq],
                                     start=(ki == 0), stop=(ki == u["nkc"] - 1))
                u["ps_c"] = ps_c

            def emit_norm(u):
                pair, rb = u["h"] // 2, (u["h"] % 2) * 64
                qo, nq, ps_c = u["qo"], u["nq"], u["ps_c"]
                rbt = rbtpool.tile([64, 512], F32, tag="rbt")
                nc.vector.reciprocal_approx_fast(rbt[:, :nq], ps_c[0:64, :nq])
                nc.vector.tensor_tensor(ctxT2[pair][rb:rb + 64, qo:qo + nq],
                                        ps_c[64:128, :nq], rbt[:, :nq], MULT)

            units = []
            for c, qo, nq in blocks:
                for h in range(HPC):
                    units.append({"c": c, "qo": qo, "nq": nq, "h": h,
                                  "kc0": koffs[c] // 128, "nkc": NKP[c] // 128})

            nvt = (NKG + 511) // 512
            vt_emitted = 0
            for i, u in enumerate(units):
                need = u["kc0"] + u["nkc"]
                while vt_emitted * 4 < need and vt_emitted < nvt:
                    emit_vtile(vt_emitted)
                    vt_emitted += 1
                emit_scores(u)
                if i >= 1:
                    emit_ctx(units[i - 1])
                if i >= 2:
                    emit_norm(units[i - 2])
            emit_ctx(units[-1])
            emit_norm(units[-2])
            emit_norm(units[-1])

        # ---- output projection ----
        NMI = (LQG + 127) // 128
        OUTR = OUT.rearrange("(n p) m -> p n m", p=128) if LQG % 128 == 0 else None
        with tc.tile_pool(name="ob", bufs=3) as obpool, \
             tc.tile_pool(name="psO", bufs=4, space="PSUM") as psO:
            oflip = True
            ob = None
            for mi in range(NMI):
                mw = min(128, LQG - mi * 128)
                if mi % 2 == 0:
                    ob = obpool.tile([128, 2, 1024], BF16, tag="ob")
                half = mi % 2
                for n2 in range(2):
                    ps_o = psO.tile([128, 512], F32, tag="ps_o")
                    for pair in range(2):
                        nc.tensor.matmul(
                            ps_o[:mw, :],
                            ctxT2[pair][:, mi * 128: mi * 128 + mw],
                            wo[:, pair, n2 * 512:(n2 + 1) * 512],
                            start=(pair == 0), stop=(pair == 1))
                    if oflip:
                        nc.vector.tensor_copy(ob[:mw, half, n2 * 512:(n2 + 1) * 512],
                                              ps_o[:mw, :])
                    else:
                        nc.scalar.copy(ob[:mw, half, n2 * 512:(n2 + 1) * 512],
                                       ps_o[:mw, :])
                    oflip = not oflip
                last = (mi == NMI - 1)
                if half == 1 or last:
                    m0 = (mi - half) * 128
                    rows = min(256 if half else 128, LQG - m0)
                    nh = half + 1
                    if OUTR is not None and rows == nh * 128:
                        nc.sync.dma_start(OUTR[:, mi - half: mi + 1, :],
                                          ob[:, 0:nh, :])
                    else:
                        for j in range(nh):
                            r = min(128, LQG - (mi - half + j) * 128)
                            nc.sync.dma_start(
                                OUT[(mi - half + j) * 128:(mi - half + j) * 128 + r, :],
                                ob[:r, j, :])

    nc.compile()
    return nc


_CACHE = {}


def run(inputs, trace=False):
    queries = np.asarray(inputs["queries"], np.float32)
    keys = np.asarray(inputs["keys"], np.float32)
    values = np.asarray(inputs["values"], np.float32)
    Wq = np.asarray(inputs["Wq"], np.float32)
    Wk = np.asarray(inputs["Wk"], np.float32)
    Wv = np.asarray(inputs["Wv"], np.float32)
    Wo = np.asarray(inputs["Wo"], np.float32)
    Wr = np.asarray(inputs["Wr"], np.float32)

    B, LQ, D_ = queries.shape
    M = Wr.shape[1]
    DH = D_ // H
    scale = np.float32(1.0 / np.sqrt(DH))
    npdt = ml_dtypes.bfloat16

    aq = np.argmax(queries @ Wr, axis=-1)   # [B, LQ]
    ak = np.argmax(keys @ Wr, axis=-1)      # [B, LK]

    NQP, NKP, qoffs, koffs, LQG, NKG = _plan(aq, ak, M)
    NVC = NKG // 128
    nk = np.array([[int((ak[b] == c).sum()) for c in range(M)] for b in range(B)])
    nkmax = [int(nk[:, c].max()) for c in range(M)]
    ckoff = np.concatenate([[0], np.cumsum(nkmax)])
    ckoffs = ckoff[:-1].tolist()
    KL = _ceil_to(int(ckoff[-1]), 4)

    key = (tuple(NQP), tuple(NKP), LQG, NKG, KL, tuple(nkmax))
    if key not in _CACHE:
        _CACHE[key] = _build_program(NQP, NKP, qoffs, koffs, ckoffs, nkmax,
                                     LQG, NKG, KL)
    nc = _CACHE[key]

    # ---- gather + zero-pad, build per-batch inputs ----
    perm_q, slot_q = [], []
    XQTs, XKTs, XVTs, MSKs = [], [], [], []
    for b in range(B):
        xq = np.zeros((LQG, D_), np.float32)
        xk = np.zeros((KL, D_), np.float32)
        xv = np.zeros((NKG, D_), np.float32)
        mska = np.zeros((NVC * 128,), np.float32)
        pq, sq = [], []
        for c in range(M):
            tq = np.nonzero(aq[b] == c)[0]
            tk = np.nonzero(ak[b] == c)[0]
            xq[qoffs[c]:qoffs[c] + len(tq)] = queries[b, tq]
            xk[ckoffs[c]:ckoffs[c] + len(tk)] = keys[b, tk]
            xv[koffs[c]:koffs[c] + len(tk)] = values[b, tk]
            mska[koffs[c]:koffs[c] + len(tk)] = 1.0
            pq.append(tq)
            sq.append(np.arange(qoffs[c], qoffs[c] + len(tq)))
        perm_q.append(np.concatenate(pq))
        slot_q.append(np.concatenate(sq))

        def swz(x):
            L = x.shape[0]
            T = (L + 511) // 512
            xp = np.zeros((T * 512, D_), np.float32)
            xp[:L] = x
            return np.ascontiguousarray(xp.T).astype(npdt)

        XQTs.append(swz(xq))
        XKTs.append(swz(xk))
        XVTs.append(swz(xv))
        # msk[p, c] = real(key at chunk c, partition p)
        MSKs.append(np.ascontiguousarray(
            mska.reshape(NVC, 128).T))

    in_maps = []
    for core in range(N_CORES):
        b, hg = core // HPC, core % HPC
        cols = slice(hg * HPC * DH, (hg + 1) * HPC * DH)
        def wswz(w):  # [1024, 256] -> [128, 8, 256]
            return np.ascontiguousarray(
                w.reshape(ND, 128, 256).transpose(1, 0, 2)).astype(npdt)

        xq0 = np.ascontiguousarray(
            XQTs[b][:, :1024].reshape(ND, 128, 2, 512).transpose(1, 2, 0, 3))
        in_maps.append({
            "XQT": XQTs[b], "XKT": XKTs[b], "XVT": XVTs[b], "XQ0": xq0,
            "WQ": wswz(Wq[:, cols] * scale),
            "WK": wswz(Wk[:, cols]),
            "WV": wswz(Wv[:, cols]),
            "WO": np.ascontiguousarray(
                Wo[cols, :].reshape(2, 128, 1024).transpose(1, 0, 2)).astype(npdt),
            "MSK": MSKs[b],
        })

    res = run_bass_kernel_spmd(nc, in_maps, list(range(N_CORES)), trace=trace)

    out = np.zeros((B, LQ, D_), np.float32)
    for b in range(B):
        acc = res.results[b * HPC]["OUT"].astype(np.float32)
        for hg in range(1, HPC):
            acc += res.results[b * HPC + hg]["OUT"].astype(np.float32)
        out[b, perm_q[b]] = acc[slot_q[b]]
    return out, res


def kernel(**inputs):
    out, _ = run(inputs)
    return out
